# revision 14
# baseline (speedup 1.0000x reference)
"""AdaptiveSparseAttention on 8 TRN2 NeuronCores.

Sharding: tensor-parallel over heads (4 heads/core) for QKV+attention,
exact-f32 router via partial matmul + AllReduce, AllToAll reshard to
token-parallel for the output projection. Host gathers 8 token shards.
"""
import sys
sys.path.insert(0, "/opt/trn_rl_repo")
import numpy as np
import concourse.bass as bass
import concourse.mybir as mybir
import concourse.tile as tile
from concourse import bacc
from concourse.bass_utils import run_bass_kernel_spmd
from concourse.masks import make_identity

DT = mybir.dt
F32 = DT.float32
BF16 = DT.bfloat16
AF = mybir.ActivationFunctionType
OP = mybir.AluOpType

NCORES = 8
B, T, D = 4, 1024, 2048
H, DH = 32, 64
HL = 4             # local heads per core
NTOK = B * T       # 4096 flattened tokens
DSL = D // NCORES  # 256: x d-slice per core
TB = 512           # token tile in pass1
NTB = NTOK // TB   # 8
KT = D // 128      # 16 k-tiles
ROPE_BASE = 10000.0


def _build():
    nc = bacc.Bacc("TRN2", target_bir_lowering=False, debug=False, num_devices=NCORES)
    x_sl = nc.dram_tensor("x_sl", [NTOK, DSL], F32, kind="ExternalInput").ap()
    w_qk = nc.dram_tensor("w_qk", [D, 512], F32, kind="ExternalInput").ap()
    w_v = nc.dram_tensor("w_v", [D, 256], F32, kind="ExternalInput").ap()
    w_r = nc.dram_tensor("w_r", [DSL, H], F32, kind="ExternalInput").ap()
    w_out = nc.dram_tensor("w_out", [D, D], F32, kind="ExternalInput").ap()
    cos4 = nc.dram_tensor("cos4", [128, NTOK], F32, kind="ExternalInput").ap()
    ssin4 = nc.dram_tensor("ssin4", [128, NTOK], F32, kind="ExternalInput").ap()
    sel = nc.dram_tensor("sel", [H, 128], F32, kind="ExternalInput").ap()
    out = nc.dram_tensor("out", [TB, D], F32, kind="ExternalOutput").ap()

    with tile.TileContext(nc) as tc:
        with (
            tc.tile_pool(name="consts", bufs=1) as consts,
            tc.tile_pool(name="persist", bufs=1) as persist,
            tc.tile_pool(name="wpool", bufs=2) as wpool,
            tc.tile_pool(name="xph", bufs=3) as xph,
            tc.tile_pool(name="stream", bufs=2) as stream,
            tc.tile_pool(name="cspool", bufs=1) as cspool,
            tc.tile_pool(name="rope", bufs=1) as rope,
            tc.tile_pool(name="rt", bufs=1) as rt,
            tc.tile_pool(name="att", bufs=2) as att,
            tc.tile_pool(name="oproj", bufs=1) as oproj,
            tc.tile_pool(name="oproj2", bufs=2) as oproj2,
            tc.tile_pool(name="ps", bufs=1, space="PSUM") as ps,
            tc.tile_pool(name="dram", bufs=1, space="DRAM") as dram,
        ):
            # ---- consts ----
            ident_b = consts.tile([128, 128], BF16)
            make_identity(nc, ident_b[:])
            ident_f = consts.tile([128, 128], F32)
            make_identity(nc, ident_f[:])
            ones_b = consts.tile([1, 64], BF16)
            nc.vector.memset(ones_b[:], 1.0)
            sel_sb = consts.tile([H, 128], F32)
            nc.sync.dma_start(sel_sb[:], sel[:])

            # ---- persistent SBUF ----
            qkT = persist.tile([128, 4, NTOK], BF16)   # [2 heads x 64, cb, tok]
            vT = persist.tile([128, 2, NTOK], BF16)    # pair per 64-row half
            gate_l = persist.tile([128, NTOK], F32)  # rows 0/32/64/96 hold local heads

            # ---- DRAM internal ----
            ag_in = [dram.tile([DSL, NTOK // 2], BF16, name=f"ag_in{_i}")
                     for _i in range(2)]
            ag_out = [dram.tile([D, NTOK // 2], BF16, name=f"ag_out{_i}")
                      for _i in range(2)]
            USE_SPLIT_AG = True
            ar_in = dram.tile([NTOK, H], F32)
            ar_out = dram.tile([NTOK, H], F32)
            a2a_in = [dram.tile([1024, TB], BF16, name=f"a2a_in{_i}") for _i in range(2)]
            a2a_out = [dram.tile([1024, TB], BF16, name=f"a2a_out{_i}") for _i in range(2)]

            # ---- weights preload (converted to bf16) ----
            wqk_sb = persist.tile([128, KT, 512], BF16)
            wv_sb = persist.tile([128, KT, 256], BF16)
            for kt in range(KT):
                wtmp = wpool.tile([128, 512], F32, tag="wtmp")
                nc.sync.dma_start(wtmp[:], w_qk[kt * 128:(kt + 1) * 128, :])
                nc.any.tensor_copy(wqk_sb[:, kt, :], wtmp[:])
                wtmp2 = wpool.tile([128, 256], F32, tag="wtmp2")
                nc.sync.dma_start(wtmp2[:], w_v[kt * 128:(kt + 1) * 128, :])
                nc.any.tensor_copy(wv_sb[:, kt, :], wtmp2[:])
            wr_sb = persist.tile([128, 2, H], F32)
            nc.sync.dma_start(wr_sb[:], w_r.rearrange("(a p) h -> p a h", p=128))

            # ---- phase 1: transpose x-slice, write AG input, router partials ----
            for tokb in range(32):
                xs = xph.tile([128, DSL], F32, tag="xs")
                nc.sync.dma_start(xs[:], x_sl[tokb * 128:(tokb + 1) * 128, :])
                ps_r = ps.tile([128, H], F32, tag="A2", name="ps_r")
                for db in range(2):
                    tp = ps.tile([128, 128], F32, tag=f"A{db}", name="tp")
                    nc.tensor.transpose(tp[:], xs[:, db * 128:(db + 1) * 128], ident_f[:])
                    hi = xph.tile([128, 128], BF16, tag="hi")
                    nc.scalar.activation(hi[:], tp[:], AF.Copy)
                    xf = xph.tile([128, 128], F32, tag="xf")
                    nc.vector.tensor_copy(xf[:], tp[:])
                    agh, agf = divmod(tokb * 128, NTOK // 2)
                    nc.sync.dma_start(
                        ag_in[agh][db * 128:(db + 1) * 128, agf:agf + 128], hi[:])
                    # router partial: [tok,32] += xf.T @ w_r  (f32, exact)
                    nc.tensor.matmul(ps_r[:], xf[:], wr_sb[:, db, :],
                                     start=(db == 0), stop=(db == 1))
                rsb = xph.tile([128, H], F32, tag="rsb")
                nc.vector.tensor_copy(rsb[:], ps_r[:])
                nc.sync.dma_start(ar_in[tokb * 128:(tokb + 1) * 128, :], rsb[:])
                if tokb == 31 or (USE_SPLIT_AG and tokb == 15):
                    for half in ([tokb // 16] if USE_SPLIT_AG else [0, 1]):
                        nc.gpsimd.collective_compute(
                            "AllGather", OP.bypass, replica_groups=[list(range(NCORES))],
                            ins=[ag_in[half].opt()], outs=[ag_out[half].opt()])

            nc.gpsimd.collective_compute(
                "AllReduce", OP.add, replica_groups=[list(range(NCORES))],
                ins=[ar_in.opt()], outs=[ar_out.opt()])

            # ---- phase 2: unified QKV^T pass ----
            for tb in range(NTB):
                tsl = slice(tb * TB, (tb + 1) * TB)
                rh = []
                for half in range(2):
                    r = stream.tile([128, 8, TB], BF16, tag="rhs", name=f"rhs{half}")
                    for kk in range(8):
                        kt = half * 8 + kk
                        aghalf, aghoff = divmod(tb * TB, NTOK // 2)
                        nc.sync.dma_start(
                            r[:, kk, :],
                            ag_out[aghalf][kt * 128:(kt + 1) * 128, aghoff:aghoff + TB])
                    rh.append(r)
                cs_c = cspool.tile([128, TB], F32, tag="cs_c")
                nc.sync.dma_start(cs_c[:], cos4[:, tsl])
                cs_s = cspool.tile([128, TB], F32, tag="cs_s")
                nc.sync.dma_start(cs_s[:], ssin4[:, tsl])
                ps_qk = [ps.tile([128, TB], F32, tag=f"A{cb}", name=f"ps_qk{cb}")
                         for cb in range(4)]
                ps_v = [ps.tile([128, TB], F32, tag=f"A{4 + vb}", name=f"ps_v{vb}")
                        for vb in range(2)]
                for kt in range(KT):
                    rhs = rh[kt // 8][:, kt % 8, :]
                    for cb in range(4):
                        nc.tensor.matmul(ps_qk[cb][:], wqk_sb[:, kt, cb * 128:(cb + 1) * 128],
                                         rhs, start=(kt == 0), stop=(kt == KT - 1))
                    for vb in range(2):
                        nc.tensor.matmul(ps_v[vb][:], wv_sb[:, kt, vb * 128:(vb + 1) * 128],
                                         rhs, start=(kt == 0), stop=(kt == KT - 1))
                # RoPE epilogue on the 4 qk blocks
                for cb in range(4):
                    csb = rope.tile([128, TB], F32, tag="C")
                    nc.scalar.activation(csb[:], ps_qk[cb][:], AF.Copy)
                    swp = rope.tile([128, TB], F32, tag="S")
                    for g in range(4):
                        sg = g ^ 1
                        nc.sync.dma_start(swp[g * 32:(g + 1) * 32, :],
                                          csb[sg * 32:(sg + 1) * 32, :])
                    t1 = rope.tile([128, TB], F32, tag="T1")
                    nc.vector.tensor_tensor(t1[:], ps_qk[cb][:], cs_c[:], OP.mult)
                    t2 = rope.tile([128, TB], F32, tag="T2")
                    nc.vector.tensor_tensor(t2[:], swp[:], cs_s[:], OP.mult)
                    nc.vector.tensor_tensor(qkT[:, cb, tsl], t1[:], t2[:], OP.add)
                for vb in range(2):
                    nc.scalar.activation(vT[:, vb, tsl], ps_v[vb][:], AF.Copy)

            # ---- phase 3: router softmax + top-4 + gate rows ----
            e = persist.tile([128, 32, H], F32)
            for tokb in range(32):
                nc.sync.dma_start(e[:, tokb, :], ar_out[tokb * 128:(tokb + 1) * 128, :])
            rmax = rt.tile([128, 32], F32, tag="rmax")
            nc.vector.tensor_reduce(rmax[:], e[:], axis=mybir.AxisListType.X, op=OP.max)
            nc.vector.tensor_tensor(e[:], e[:], rmax[:, :, None].to_broadcast((128, 32, H)),
                                    OP.subtract)
            nc.scalar.activation(e[:].rearrange("p a h -> p (a h)"),
                                 e[:].rearrange("p a h -> p (a h)"), AF.Exp)
            ssum = rt.tile([128, 32], F32, tag="ssum")
            nc.vector.tensor_reduce(ssum[:], e[:], axis=mybir.AxisListType.X, op=OP.add)
            rs = rt.tile([128, 32], F32, tag="rs")
            nc.vector.reciprocal(rs[:], ssum[:])
            ecur = persist.tile([128, 32, H], F32)
            nc.vector.tensor_copy(ecur[:], e[:])
            ge = rt.tile([128, 32, H], F32, tag="ge")
            for it in range(4):
                m = rt.tile([128, 32], F32, tag="m")
                nc.vector.tensor_reduce(m[:], ecur[:], axis=mybir.AxisListType.X, op=OP.max)
                nc.vector.tensor_tensor(ge[:], ecur[:],
                                        m[:, :, None].to_broadcast((128, 32, H)), OP.is_ge)
                nc.vector.scalar_tensor_tensor(ecur[:], ge[:], -1e9, ecur[:],
                                               OP.mult, OP.add)
            mask = rt.tile([128, 32, H], F32, tag="mask")
            nc.vector.tensor_scalar(mask[:], ecur[:], -1e6, None, OP.is_lt)
            gate = persist.tile([128, 32, H], F32)
            nc.vector.tensor_tensor(gate[:], e[:], mask[:], OP.mult)
            nc.vector.tensor_tensor(gate[:], gate[:],
                                    rs[:, :, None].to_broadcast((128, 32, H)), OP.mult)
            # extract the 4 local head rows (sel is the per-core one-hot [32, 4])
            for tokb in range(32):
                gt_ps = ps.tile([H, 128], F32, tag="A6", name="gt_ps")
                nc.tensor.transpose(gt_ps[:], gate[:, tokb, :], ident_f[:])
                gt_sb = rt.tile([H, 128], F32, tag="gt_sb")
                nc.vector.tensor_copy(gt_sb[:], gt_ps[:])
                g4_ps = ps.tile([128, 128], F32, tag="A7", name="g4_ps")
                nc.tensor.matmul(g4_ps[:], sel_sb[:], gt_sb[:], start=True, stop=True)
                for l in range(HL):
                    nc.vector.tensor_copy(
                        gate_l[32 * l:32 * l + 1, tokb * 128:(tokb + 1) * 128],
                        g4_ps[32 * l:32 * l + 1, :])

            # ---- phase 4: attention per (head-pair, batch) ----
            for hp in range(2):
                for b in range(B):
                    bt = b * T
                    va = [att.tile([128, 8, 72], BF16, tag=f"va{hl}", name=f"va{hl}")
                          for hl in range(2)]
                    for hl in range(2):
                        base = hl * 64
                        idn = ident_b[base:base + 64, base:base + 64]
                        for tkb in range(8):
                            vps = ps.tile([128, 64], BF16, tag="A6", name="vps")
                            nc.tensor.transpose(
                                vps[:],
                                vT[base:base + 64, hp, bt + tkb * 128:bt + (tkb + 1) * 128],
                                idn)
                            nc.any.tensor_copy(va[hl][:, tkb, 0:64], vps[:])
                            nc.vector.memset(va[hl][:, tkb, 64:65], 1.0)
                    for tqt in range(2):
                        qsl = slice(bt + tqt * TB, bt + (tqt + 1) * TB)
                        ntk = 4 + 4 * tqt
                        o_ps = [ps.tile([65, TB], F32, tag=f"A{4 + hl}", name=f"o_ps{hl}")
                                for hl in range(2)]
                        for tkb in range(ntk):
                            ksl = slice(bt + tkb * 128, bt + (tkb + 1) * 128)
                            s_ps = [ps.tile([128, TB], F32, tag=f"A{2 * hl + tkb % 2}",
                                            name=f"s_ps{hl}") for hl in range(2)]
                            nc.tensor.matmul(s_ps[0][:], qkT[0:64, 2 + hp, ksl],
                                             qkT[0:64, hp, qsl], start=True, stop=True,
                                             tile_position=(0, 0))
                            nc.tensor.matmul(s_ps[1][:], qkT[64:128, 2 + hp, ksl],
                                             qkT[64:128, hp, qsl], start=True, stop=True,
                                             tile_position=(64, 0))
                            dd = tqt * 4 - tkb
                            for hl in range(2):
                                p_sb = att.tile([128, TB], BF16, tag=f"p{hl}", name=f"p{hl}")
                                if dd >= 1:
                                    nc.scalar.activation(p_sb[:], s_ps[hl][:], AF.Exp,
                                                         scale=0.125)
                                else:
                                    off = -dd * 128
                                    if off > 0:
                                        nc.vector.memset(p_sb[:, 0:off], 0.0)
                                    nc.scalar.activation(p_sb[:, off:TB], s_ps[hl][:, off:TB],
                                                         AF.Exp, scale=0.125)
                                    nc.gpsimd.affine_select(
                                        out=p_sb[:, off:off + 128], in_=p_sb[:, off:off + 128],
                                        compare_op=OP.is_ge, fill=0.0,
                                        base=0, pattern=[[1, 128]], channel_multiplier=-1)
                                nc.tensor.matmul(o_ps[hl][:], va[hl][:, tkb, 0:65], p_sb[:],
                                                 start=(tkb == 0), stop=(tkb == ntk - 1))
                        for hl in range(2):
                            l = 2 * hp + hl
                            grow = att.tile([1, TB], F32, tag="grow")
                            nc.vector.tensor_copy(grow[:], gate_l[32 * l:32 * l + 1, qsl])
                            recip = att.tile([1, TB], F32, tag="recip")
                            nc.vector.reciprocal(recip[:], o_ps[hl][64:65, :])
                            scale_sb = att.tile([1, TB], BF16, tag="scale")
                            nc.vector.tensor_tensor(scale_sb[:], grow[:], recip[:], OP.mult)
                            bc_ps = ps.tile([64, TB], F32, tag="A6", name="bc_ps")
                            nc.tensor.matmul(bc_ps[:], ones_b[:], scale_sb[:],
                                             start=True, stop=True)
                            bc_sb = att.tile([64, TB], F32, tag="bc_sb")
                            nc.scalar.activation(bc_sb[:], bc_ps[:], AF.Copy)
                            oT = att.tile([64, TB], BF16, tag="oT_sb")
                            nc.vector.tensor_tensor(oT[:], o_ps[hl][0:64, :], bc_sb[:], OP.mult)
                            j = 2 * b + tqt
                            nc.sync.dma_start(
                                a2a_in[hp][j * 128 + hl * 64:j * 128 + (hl + 1) * 64, :],
                                oT[:])
                nc.gpsimd.collective_compute(
                    "AllToAll", OP.bypass, replica_groups=[list(range(NCORES))],
                    ins=[a2a_in[hp].opt()], outs=[a2a_out[hp].opt()])

            # ---- phase 5: output projection on my token shard ----
            for hp in range(2):
                rcv = [oproj.tile([128, TB], BF16, tag=f"rcv{i}", name=f"rcv{i}")
                       for i in range(8)]
                for i in range(8):
                    nc.sync.dma_start(rcv[i][:], a2a_out[hp][i * 128:(i + 1) * 128, :])
                for n in range(4):
                    wo = [oproj.tile([128, 512], BF16, tag=f"wo{i}", name=f"wo{i}")
                          for i in range(8)]
                    for i in range(8):
                        wof = oproj2.tile([128, 512], F32, tag="wof")
                        nc.sync.dma_start(
                            wof[:], w_out[i * 256 + hp * 128:i * 256 + (hp + 1) * 128,
                                          n * 512:(n + 1) * 512])
                        nc.any.tensor_copy(wo[i][:], wof[:])
                    for m_ in range(4):
                        op_ps = ps.tile([128, 512], F32, tag="A7", name="op_ps")
                        for i in range(8):
                            nc.tensor.matmul(op_ps[:], rcv[i][:, m_ * 128:(m_ + 1) * 128],
                                             wo[i][:], start=(i == 0), stop=(i == 7))
                        ostage = oproj2.tile([128, 512], F32, tag="ostage")
                        nc.vector.tensor_copy(ostage[:], op_ps[:])
                        r0 = slice(m_ * 128, (m_ + 1) * 128)
                        c0 = slice(n * 512, (n + 1) * 512)
                        if hp == 0:
                            nc.sync.dma_start(out[r0, c0], ostage[:])
                        else:
                            nc.gpsimd.dma_start(out[r0, c0], ostage[:], accum_op=OP.add)

    nc.compile()
    return nc


_CACHE = {}


def _get_nc():
    if "nc" not in _CACHE:
        _CACHE["nc"] = _build()
    return _CACHE["nc"]


def _host_inputs(x, w_router, w_qkv, w_out):
    x2 = np.ascontiguousarray(np.asarray(x, dtype=np.float32).reshape(NTOK, D))
    w_qkv = np.asarray(w_qkv, dtype=np.float32)
    w_router = np.asarray(w_router, dtype=np.float32)
    w_out = np.ascontiguousarray(np.asarray(w_out, dtype=np.float32))

    # RoPE tables, de-interleaved layout
    invf = 1.0 / (ROPE_BASE ** (np.arange(0, DH, 2, dtype=np.float32) / DH))  # [32]
    tt = np.arange(NTOK, dtype=np.float32) % T
    ang = tt[None, :] * invf[:, None]
    cos1 = np.cos(ang).astype(np.float32)
    sin1 = np.sin(ang).astype(np.float32)
    cos4 = np.ascontiguousarray(np.tile(cos1, (4, 1)).astype(np.float32))
    ssin4 = np.ascontiguousarray(
        np.concatenate([-sin1, sin1, -sin1, sin1], axis=0).astype(np.float32))

    in_maps = []
    for c in range(NCORES):
        heads = [4 * c + i for i in range(HL)]

        def deint(h, base):
            cols = np.arange(h * DH, (h + 1) * DH)
            return np.concatenate([base + cols[0::2], base + cols[1::2]])

        qk_cols = np.concatenate(
            [deint(heads[0], 0), deint(heads[1], 0),
             deint(heads[2], 0), deint(heads[3], 0),
             deint(heads[0], D), deint(heads[1], D),
             deint(heads[2], D), deint(heads[3], D)])
        v_cols = np.concatenate([2 * D + np.arange(h * DH, (h + 1) * DH) for h in heads])
        sel_np = np.zeros((H, 128), dtype=np.float32)
        for l in range(HL):
            sel_np[4 * c + l, 32 * l] = 1.0
        in_maps.append({
            "x_sl": np.ascontiguousarray(x2[:, c * DSL:(c + 1) * DSL]),
            "w_qk": np.ascontiguousarray(w_qkv[:, qk_cols]),
            "w_v": np.ascontiguousarray(w_qkv[:, v_cols]),
            "w_r": np.ascontiguousarray(w_router[c * DSL:(c + 1) * DSL, :]),
            "w_out": w_out,
            "cos4": cos4,
            "ssin4": ssin4,
            "sel": sel_np,
        })
    return in_maps


def run(x, w_router, w_qkv, w_out, trace=False):
    nc = _get_nc()
    in_maps = _host_inputs(x, w_router, w_qkv, w_out)
    res = run_bass_kernel_spmd(nc, in_maps, core_ids=list(range(NCORES)), trace=trace)
    shards = [res.results[c]["out"] for c in range(NCORES)]
    full = np.concatenate(shards, axis=0).reshape(B, T, D).astype(np.float32)
    return full, res


def kernel(x, w_router, w_qkv, w_out):
    full, _ = run(x, w_router, w_qkv, w_out, trace=False)
    return full


# revision 17
# speedup vs baseline: 1.1654x; 1.1654x over previous
"""AdaptiveSparseAttention on 8 TRN2 NeuronCores.

Sharding: tensor-parallel over heads (4 heads/core) for QKV+attention,
exact-f32 router via partial matmul + AllReduce, AllToAll reshard to
token-parallel for the output projection. Host gathers 8 token shards.
"""
import sys
sys.path.insert(0, "/opt/trn_rl_repo")
import numpy as np
import concourse.bass as bass
import concourse.mybir as mybir
import concourse.tile as tile
from concourse import bacc
from concourse.bass_utils import run_bass_kernel_spmd
from concourse.masks import make_identity

DT = mybir.dt
F32 = DT.float32
BF16 = DT.bfloat16
AF = mybir.ActivationFunctionType
OP = mybir.AluOpType

NCORES = 8
B, T, D = 4, 1024, 2048
H, DH = 32, 64
HL = 4             # local heads per core
NTOK = B * T       # 4096 flattened tokens
DSL = D // NCORES  # 256: x d-slice per core
TB = 512           # token tile in pass1
NTB = NTOK // TB   # 8
KT = D // 128      # 16 k-tiles
ROPE_BASE = 10000.0


def _build():
    nc = bacc.Bacc("TRN2", target_bir_lowering=False, debug=False, num_devices=NCORES)
    x_sl = nc.dram_tensor("x_sl", [NTOK, DSL], F32, kind="ExternalInput").ap()
    w_qk = nc.dram_tensor("w_qk", [D, 512], F32, kind="ExternalInput").ap()
    w_v = nc.dram_tensor("w_v", [D, 256], F32, kind="ExternalInput").ap()
    w_r = nc.dram_tensor("w_r", [DSL, H], F32, kind="ExternalInput").ap()
    w_out = nc.dram_tensor("w_out", [D, D], F32, kind="ExternalInput").ap()
    cos4 = nc.dram_tensor("cos4", [128, NTOK], F32, kind="ExternalInput").ap()
    ssin4 = nc.dram_tensor("ssin4", [128, NTOK], F32, kind="ExternalInput").ap()
    sel = nc.dram_tensor("sel", [H, 128], F32, kind="ExternalInput").ap()
    out = nc.dram_tensor("out", [TB, D], F32, kind="ExternalOutput").ap()

    with tile.TileContext(nc) as tc:
        with (
            tc.tile_pool(name="consts", bufs=1) as consts,
            tc.tile_pool(name="persist", bufs=1) as persist,
            tc.tile_pool(name="wpool", bufs=2) as wpool,
            tc.tile_pool(name="xph", bufs=3) as xph,
            tc.tile_pool(name="stream", bufs=2) as stream,
            tc.tile_pool(name="cspool", bufs=1) as cspool,
            tc.tile_pool(name="rope", bufs=1) as rope,
            tc.tile_pool(name="rt", bufs=1) as rt,
            tc.tile_pool(name="att", bufs=2) as att,
            tc.tile_pool(name="oproj", bufs=1) as oproj,
            tc.tile_pool(name="oproj2", bufs=2) as oproj2,
            tc.tile_pool(name="ps", bufs=1, space="PSUM") as ps,
            tc.tile_pool(name="dram", bufs=1, space="DRAM") as dram,
        ):
            # ---- consts ----
            ident_b = consts.tile([128, 128], BF16)
            make_identity(nc, ident_b[:])
            ident_f = consts.tile([128, 128], F32)
            make_identity(nc, ident_f[:])
            ones_b = consts.tile([1, 64], BF16)
            nc.vector.memset(ones_b[:], 1.0)
            sel_sb = consts.tile([H, 128], F32)
            nc.sync.dma_start(sel_sb[:], sel[:])

            # ---- persistent SBUF ----
            qkT = persist.tile([128, 4, NTOK], BF16)   # [2 heads x 64, cb, tok]
            vT = persist.tile([128, 2, NTOK], BF16)    # pair per 64-row half
            gate_l = persist.tile([128, NTOK], F32)  # rows 0/32/64/96 hold local heads

            # ---- DRAM internal ----
            ag_in = [dram.tile([DSL, NTOK // 2], BF16, name=f"ag_in{_i}")
                     for _i in range(2)]
            ag_out = [dram.tile([D, NTOK // 2], BF16, name=f"ag_out{_i}")
                      for _i in range(2)]
            USE_SPLIT_AG = True
            ar_in = dram.tile([NTOK, H], F32)
            ar_out = dram.tile([NTOK, H], F32)
            a2a_in = [dram.tile([1024, TB], BF16, name=f"a2a_in{_i}") for _i in range(2)]
            a2a_out = [dram.tile([1024, TB], BF16, name=f"a2a_out{_i}") for _i in range(2)]

            # ---- weights preload (converted to bf16) ----
            wqk_sb = persist.tile([128, KT * 4, 128], BF16)
            wv_sb = persist.tile([128, KT * 2, 128], BF16)
            for kt in range(KT):
                wtmp = wpool.tile([128, 512], F32, tag="wtmp")
                nc.sync.dma_start(wtmp[:], w_qk[kt * 128:(kt + 1) * 128, :])
                nc.any.tensor_copy(
                    wqk_sb[:, kt * 4:(kt + 1) * 4, :].rearrange("p a b -> p (a b)"),
                    wtmp[:])
                wtmp2 = wpool.tile([128, 256], F32, tag="wtmp2")
                nc.sync.dma_start(wtmp2[:], w_v[kt * 128:(kt + 1) * 128, :])
                nc.any.tensor_copy(
                    wv_sb[:, kt * 2:(kt + 1) * 2, :].rearrange("p a b -> p (a b)"),
                    wtmp2[:])
            wr_sb = persist.tile([128, 2, H], F32)
            nc.sync.dma_start(wr_sb[:], w_r.rearrange("(a p) h -> p a h", p=128))

            # ---- phase 1: transpose x-slice, write AG input, router partials ----
            for tokb in range(32):
                xs = xph.tile([128, DSL], F32, tag="xs")
                nc.sync.dma_start(xs[:], x_sl[tokb * 128:(tokb + 1) * 128, :])
                ps_r = ps.tile([128, H], F32, tag="A2", name="ps_r")
                for db in range(2):
                    tp = ps.tile([128, 128], F32, tag=f"A{db}", name="tp")
                    nc.tensor.transpose(tp[:], xs[:, db * 128:(db + 1) * 128], ident_f[:])
                    hi = xph.tile([128, 128], BF16, tag="hi")
                    nc.scalar.activation(hi[:], tp[:], AF.Copy)
                    xf = xph.tile([128, 128], F32, tag="xf")
                    nc.vector.tensor_copy(xf[:], tp[:])
                    agh, agf = divmod(tokb * 128, NTOK // 2)
                    nc.sync.dma_start(
                        ag_in[agh][db * 128:(db + 1) * 128, agf:agf + 128], hi[:])
                    # router partial: [tok,32] += xf.T @ w_r  (f32, exact)
                    nc.tensor.matmul(ps_r[:], xf[:], wr_sb[:, db, :],
                                     start=(db == 0), stop=(db == 1))
                rsb = xph.tile([128, H], F32, tag="rsb")
                nc.vector.tensor_copy(rsb[:], ps_r[:])
                nc.sync.dma_start(ar_in[tokb * 128:(tokb + 1) * 128, :], rsb[:])
                if tokb == 31 or (USE_SPLIT_AG and tokb == 15):
                    for half in ([tokb // 16] if USE_SPLIT_AG else [0, 1]):
                        nc.gpsimd.collective_compute(
                            "AllGather", OP.bypass, replica_groups=[list(range(NCORES))],
                            ins=[ag_in[half].opt()], outs=[ag_out[half].opt()])

            nc.gpsimd.collective_compute(
                "AllReduce", OP.add, replica_groups=[list(range(NCORES))],
                ins=[ar_in.opt()], outs=[ar_out.opt()])

            # ---- phase 2: unified QKV^T pass ----
            for tb in range(NTB):
                tsl = slice(tb * TB, (tb + 1) * TB)
                rh = []
                for half in range(2):
                    r = stream.tile([128, 8, TB], BF16, tag="rhs", name=f"rhs{half}")
                    for kk in range(8):
                        kt = half * 8 + kk
                        aghalf, aghoff = divmod(tb * TB, NTOK // 2)
                        nc.sync.dma_start(
                            r[:, kk, :],
                            ag_out[aghalf][kt * 128:(kt + 1) * 128, aghoff:aghoff + TB])
                    rh.append(r)
                cs_c = cspool.tile([128, TB], F32, tag="cs_c")
                nc.sync.dma_start(cs_c[:], cos4[:, tsl])
                cs_s = cspool.tile([128, TB], F32, tag="cs_s")
                nc.sync.dma_start(cs_s[:], ssin4[:, tsl])
                ps_qk = [ps.tile([128, TB], F32, tag=f"A{cb}", name=f"ps_qk{cb}")
                         for cb in range(4)]
                ps_v = [ps.tile([128, TB], F32, tag=f"A{4 + vb}", name=f"ps_v{vb}")
                        for vb in range(2)]
                for kt in range(KT):
                    rhs = rh[kt // 8][:, kt % 8, :]
                    for cb in range(4):
                        nc.tensor.matmul(ps_qk[cb][:], wqk_sb[:, kt * 4 + cb, :],
                                         rhs, start=(kt == 0), stop=(kt == KT - 1))
                    for vb in range(2):
                        nc.tensor.matmul(ps_v[vb][:], wv_sb[:, kt * 2 + vb, :],
                                         rhs, start=(kt == 0), stop=(kt == KT - 1))
                # RoPE epilogue on the 4 qk blocks
                for cb in range(4):
                    csb = rope.tile([128, TB], F32, tag="C")
                    nc.scalar.activation(csb[:], ps_qk[cb][:], AF.Copy)
                    swp = rope.tile([128, TB], F32, tag="S")
                    for g in range(4):
                        sg = g ^ 1
                        nc.sync.dma_start(swp[g * 32:(g + 1) * 32, :],
                                          csb[sg * 32:(sg + 1) * 32, :])
                    t1 = rope.tile([128, TB], F32, tag="T1")
                    nc.vector.tensor_tensor(t1[:], ps_qk[cb][:], cs_c[:], OP.mult)
                    t2 = rope.tile([128, TB], F32, tag="T2")
                    nc.vector.tensor_tensor(t2[:], swp[:], cs_s[:], OP.mult)
                    nc.vector.tensor_tensor(qkT[:, cb, tsl], t1[:], t2[:], OP.add)
                for vb in range(2):
                    nc.scalar.activation(vT[:, vb, tsl], ps_v[vb][:], AF.Copy)

            # ---- phase 3: router softmax + top-4 + gate rows ----
            e = persist.tile([128, 32, H], F32)
            for tokb in range(32):
                nc.sync.dma_start(e[:, tokb, :], ar_out[tokb * 128:(tokb + 1) * 128, :])
            rmax = rt.tile([128, 32], F32, tag="rmax")
            nc.vector.tensor_reduce(rmax[:], e[:], axis=mybir.AxisListType.X, op=OP.max)
            nc.vector.tensor_tensor(e[:], e[:], rmax[:, :, None].to_broadcast((128, 32, H)),
                                    OP.subtract)
            nc.scalar.activation(e[:].rearrange("p a h -> p (a h)"),
                                 e[:].rearrange("p a h -> p (a h)"), AF.Exp)
            ssum = rt.tile([128, 32], F32, tag="ssum")
            nc.vector.tensor_reduce(ssum[:], e[:], axis=mybir.AxisListType.X, op=OP.add)
            rs = rt.tile([128, 32], F32, tag="rs")
            nc.vector.reciprocal(rs[:], ssum[:])
            ecur = persist.tile([128, 32, H], F32)
            nc.vector.tensor_copy(ecur[:], e[:])
            ge = rt.tile([128, 32, H], F32, tag="ge")
            for it in range(4):
                m = rt.tile([128, 32], F32, tag="m")
                nc.vector.tensor_reduce(m[:], ecur[:], axis=mybir.AxisListType.X, op=OP.max)
                nc.vector.tensor_tensor(ge[:], ecur[:],
                                        m[:, :, None].to_broadcast((128, 32, H)), OP.is_ge)
                nc.vector.scalar_tensor_tensor(ecur[:], ge[:], -1e9, ecur[:],
                                               OP.mult, OP.add)
            mask = rt.tile([128, 32, H], F32, tag="mask")
            nc.vector.tensor_scalar(mask[:], ecur[:], -1e6, None, OP.is_lt)
            gate = persist.tile([128, 32, H], F32)
            nc.vector.tensor_tensor(gate[:], e[:], mask[:], OP.mult)
            nc.vector.tensor_tensor(gate[:], gate[:],
                                    rs[:, :, None].to_broadcast((128, 32, H)), OP.mult)
            # extract the 4 local head rows (sel is the per-core one-hot [32, 4])
            for tokb in range(32):
                gt_ps = ps.tile([H, 128], F32, tag="A6", name="gt_ps")
                nc.tensor.transpose(gt_ps[:], gate[:, tokb, :], ident_f[:])
                gt_sb = rt.tile([H, 128], F32, tag="gt_sb")
                nc.vector.tensor_copy(gt_sb[:], gt_ps[:])
                g4_ps = ps.tile([128, 128], F32, tag="A7", name="g4_ps")
                nc.tensor.matmul(g4_ps[:], sel_sb[:], gt_sb[:], start=True, stop=True)
                for l in range(HL):
                    nc.vector.tensor_copy(
                        gate_l[32 * l:32 * l + 1, tokb * 128:(tokb + 1) * 128],
                        g4_ps[32 * l:32 * l + 1, :])

            # ---- phase 4: attention per (head-pair, batch) ----
            for hp in range(2):
                for b in range(B):
                    bt = b * T
                    va = [att.tile([128, 8, 72], BF16, tag=f"va{hl}", name=f"va{hl}")
                          for hl in range(2)]
                    for hl in range(2):
                        base = hl * 64
                        idn = ident_b[base:base + 64, base:base + 64]
                        for tkb in range(8):
                            vps = ps.tile([128, 64], BF16, tag="A6", name="vps")
                            nc.tensor.transpose(
                                vps[:],
                                vT[base:base + 64, hp, bt + tkb * 128:bt + (tkb + 1) * 128],
                                idn)
                            nc.any.tensor_copy(va[hl][:, tkb, 0:64], vps[:])
                            nc.vector.memset(va[hl][:, tkb, 64:65], 1.0)
                    for tqt in range(2):
                        qsl = slice(bt + tqt * TB, bt + (tqt + 1) * TB)
                        ntk = 4 + 4 * tqt
                        o_ps = [ps.tile([65, TB], F32, tag=f"A{4 + hl}", name=f"o_ps{hl}")
                                for hl in range(2)]
                        for tkb in range(ntk):
                            ksl = slice(bt + tkb * 128, bt + (tkb + 1) * 128)
                            s_ps = [ps.tile([128, TB], F32, tag=f"A{2 * hl + tkb % 2}",
                                            name=f"s_ps{hl}") for hl in range(2)]
                            nc.tensor.matmul(s_ps[0][:], qkT[0:64, 2 + hp, ksl],
                                             qkT[0:64, hp, qsl], start=True, stop=True,
                                             tile_position=(0, 0))
                            nc.tensor.matmul(s_ps[1][:], qkT[64:128, 2 + hp, ksl],
                                             qkT[64:128, hp, qsl], start=True, stop=True,
                                             tile_position=(64, 0))
                            dd = tqt * 4 - tkb
                            for hl in range(2):
                                p_sb = att.tile([128, TB], BF16, tag=f"p{hl}", name=f"p{hl}")
                                if dd >= 1:
                                    nc.scalar.activation(p_sb[:], s_ps[hl][:], AF.Exp,
                                                         scale=0.125)
                                else:
                                    off = -dd * 128
                                    if off > 0:
                                        nc.vector.memset(p_sb[:, 0:off], 0.0)
                                    nc.scalar.activation(p_sb[:, off:TB], s_ps[hl][:, off:TB],
                                                         AF.Exp, scale=0.125)
                                    nc.gpsimd.affine_select(
                                        out=p_sb[:, off:off + 128], in_=p_sb[:, off:off + 128],
                                        compare_op=OP.is_ge, fill=0.0,
                                        base=0, pattern=[[1, 128]], channel_multiplier=-1)
                                nc.tensor.matmul(o_ps[hl][:], va[hl][:, tkb, 0:65], p_sb[:],
                                                 start=(tkb == 0), stop=(tkb == ntk - 1))
                        for hl in range(2):
                            l = 2 * hp + hl
                            grow = att.tile([1, TB], F32, tag="grow")
                            nc.vector.tensor_copy(grow[:], gate_l[32 * l:32 * l + 1, qsl])
                            dn_sb = att.tile([1, TB], F32, tag="dn_sb")
                            nc.scalar.activation(dn_sb[:], o_ps[hl][64:65, :], AF.Copy)
                            recip = att.tile([1, TB], F32, tag="recip")
                            nc.vector.reciprocal_approx_fast(recip[:], dn_sb[:])
                            scale_sb = att.tile([1, TB], BF16, tag="scale")
                            nc.vector.tensor_tensor(scale_sb[:], grow[:], recip[:], OP.mult)
                            bc_ps = ps.tile([64, TB], F32, tag="A6", name="bc_ps")
                            nc.tensor.matmul(bc_ps[:], ones_b[:], scale_sb[:],
                                             start=True, stop=True)
                            bc_sb = att.tile([64, TB], F32, tag="bc_sb")
                            nc.scalar.activation(bc_sb[:], bc_ps[:], AF.Copy)
                            oT = att.tile([64, TB], BF16, tag="oT_sb")
                            nc.vector.tensor_tensor(oT[:], o_ps[hl][0:64, :], bc_sb[:], OP.mult)
                            j = 2 * b + tqt
                            nc.sync.dma_start(
                                a2a_in[hp][j * 128 + hl * 64:j * 128 + (hl + 1) * 64, :],
                                oT[:])
                nc.gpsimd.collective_compute(
                    "AllToAll", OP.bypass, replica_groups=[list(range(NCORES))],
                    ins=[a2a_in[hp].opt()], outs=[a2a_out[hp].opt()])

            # ---- phase 5: output projection on my token shard ----
            for hp in range(2):
                rcv = [oproj.tile([128, TB], BF16, tag=f"rcv{i}", name=f"rcv{i}")
                       for i in range(8)]
                for i in range(8):
                    nc.sync.dma_start(rcv[i][:], a2a_out[hp][i * 128:(i + 1) * 128, :])
                for n in range(4):
                    wo = [oproj.tile([128, 512], BF16, tag=f"wo{i}", name=f"wo{i}")
                          for i in range(8)]
                    for i in range(8):
                        wof = oproj2.tile([128, 512], F32, tag="wof")
                        nc.sync.dma_start(
                            wof[:], w_out[i * 256 + hp * 128:i * 256 + (hp + 1) * 128,
                                          n * 512:(n + 1) * 512])
                        nc.any.tensor_copy(wo[i][:], wof[:])
                    for m_ in range(4):
                        op_ps = ps.tile([128, 512], F32, tag="A7", name="op_ps")
                        for i in range(8):
                            nc.tensor.matmul(op_ps[:], rcv[i][:, m_ * 128:(m_ + 1) * 128],
                                             wo[i][:], start=(i == 0), stop=(i == 7))
                        ostage = oproj2.tile([128, 512], F32, tag="ostage")
                        nc.vector.tensor_copy(ostage[:], op_ps[:])
                        r0 = slice(m_ * 128, (m_ + 1) * 128)
                        c0 = slice(n * 512, (n + 1) * 512)
                        if hp == 0:
                            nc.sync.dma_start(out[r0, c0], ostage[:])
                        else:
                            nc.gpsimd.dma_start(out[r0, c0], ostage[:], accum_op=OP.add)

    nc.compile()
    return nc


_CACHE = {}


def _get_nc():
    if "nc" not in _CACHE:
        _CACHE["nc"] = _build()
    return _CACHE["nc"]


def _host_inputs(x, w_router, w_qkv, w_out):
    x2 = np.ascontiguousarray(np.asarray(x, dtype=np.float32).reshape(NTOK, D))
    w_qkv = np.asarray(w_qkv, dtype=np.float32)
    w_router = np.asarray(w_router, dtype=np.float32)
    w_out = np.ascontiguousarray(np.asarray(w_out, dtype=np.float32))

    # RoPE tables, de-interleaved layout
    invf = 1.0 / (ROPE_BASE ** (np.arange(0, DH, 2, dtype=np.float32) / DH))  # [32]
    tt = np.arange(NTOK, dtype=np.float32) % T
    ang = tt[None, :] * invf[:, None]
    cos1 = np.cos(ang).astype(np.float32)
    sin1 = np.sin(ang).astype(np.float32)
    cos4 = np.ascontiguousarray(np.tile(cos1, (4, 1)).astype(np.float32))
    ssin4 = np.ascontiguousarray(
        np.concatenate([-sin1, sin1, -sin1, sin1], axis=0).astype(np.float32))

    in_maps = []
    for c in range(NCORES):
        heads = [4 * c + i for i in range(HL)]

        def deint(h, base):
            cols = np.arange(h * DH, (h + 1) * DH)
            return np.concatenate([base + cols[0::2], base + cols[1::2]])

        qk_cols = np.concatenate(
            [deint(heads[0], 0), deint(heads[1], 0),
             deint(heads[2], 0), deint(heads[3], 0),
             deint(heads[0], D), deint(heads[1], D),
             deint(heads[2], D), deint(heads[3], D)])
        v_cols = np.concatenate([2 * D + np.arange(h * DH, (h + 1) * DH) for h in heads])
        sel_np = np.zeros((H, 128), dtype=np.float32)
        for l in range(HL):
            sel_np[4 * c + l, 32 * l] = 1.0
        in_maps.append({
            "x_sl": np.ascontiguousarray(x2[:, c * DSL:(c + 1) * DSL]),
            "w_qk": np.ascontiguousarray(w_qkv[:, qk_cols]),
            "w_v": np.ascontiguousarray(w_qkv[:, v_cols]),
            "w_r": np.ascontiguousarray(w_router[c * DSL:(c + 1) * DSL, :]),
            "w_out": w_out,
            "cos4": cos4,
            "ssin4": ssin4,
            "sel": sel_np,
        })
    return in_maps


def run(x, w_router, w_qkv, w_out, trace=False):
    nc = _get_nc()
    in_maps = _host_inputs(x, w_router, w_qkv, w_out)
    res = run_bass_kernel_spmd(nc, in_maps, core_ids=list(range(NCORES)), trace=trace)
    shards = [res.results[c]["out"] for c in range(NCORES)]
    full = np.concatenate(shards, axis=0).reshape(B, T, D).astype(np.float32)
    return full, res


def kernel(x, w_router, w_qkv, w_out):
    full, _ = run(x, w_router, w_qkv, w_out, trace=False)
    return full


# revision 23
# speedup vs baseline: 1.1818x; 1.0141x over previous
"""AdaptiveSparseAttention on 8 TRN2 NeuronCores.

Sharding: tensor-parallel over heads (4 heads/core) for QKV+attention,
exact-f32 router via partial matmul + AllReduce, AllToAll reshard to
token-parallel for the output projection. Host gathers 8 token shards.
"""
import sys
sys.path.insert(0, "/opt/trn_rl_repo")
import numpy as np
import concourse.bass as bass
import concourse.mybir as mybir
import concourse.tile as tile
from concourse import bacc
from concourse.bass_utils import run_bass_kernel_spmd
from concourse.masks import make_identity

DT = mybir.dt
F32 = DT.float32
BF16 = DT.bfloat16
AF = mybir.ActivationFunctionType
OP = mybir.AluOpType

NCORES = 8
B, T, D = 4, 1024, 2048
H, DH = 32, 64
HL = 4             # local heads per core
NTOK = B * T       # 4096 flattened tokens
DSL = D // NCORES  # 256: x d-slice per core
TB = 512           # token tile in pass1
NTB = NTOK // TB   # 8
KT = D // 128      # 16 k-tiles
ROPE_BASE = 10000.0


def _build():
    nc = bacc.Bacc("TRN2", target_bir_lowering=False, debug=False, num_devices=NCORES)
    x_sl = nc.dram_tensor("x_sl", [NTOK, DSL], F32, kind="ExternalInput").ap()
    w_qk = nc.dram_tensor("w_qk", [D, 512], F32, kind="ExternalInput").ap()
    w_v = nc.dram_tensor("w_v", [D, 256], F32, kind="ExternalInput").ap()
    w_r = nc.dram_tensor("w_r", [DSL, H], F32, kind="ExternalInput").ap()
    w_out = nc.dram_tensor("w_out", [D, D], F32, kind="ExternalInput").ap()
    cos4 = nc.dram_tensor("cos4", [128, NTOK], F32, kind="ExternalInput").ap()
    ssin4 = nc.dram_tensor("ssin4", [128, NTOK], F32, kind="ExternalInput").ap()
    sel = nc.dram_tensor("sel", [H, 128], F32, kind="ExternalInput").ap()
    out = nc.dram_tensor("out", [TB, D], F32, kind="ExternalOutput").ap()

    with tile.TileContext(nc) as tc:
        with (
            tc.tile_pool(name="consts", bufs=1) as consts,
            tc.tile_pool(name="persist", bufs=1) as persist,
            tc.tile_pool(name="wpool", bufs=1) as wpool,
            tc.tile_pool(name="xph", bufs=3) as xph,
            tc.tile_pool(name="stream", bufs=2) as stream,
            tc.tile_pool(name="cspool", bufs=1) as cspool,
            tc.tile_pool(name="rope", bufs=1) as rope,
            tc.tile_pool(name="rt", bufs=1) as rt,
            tc.tile_pool(name="att", bufs=2) as att,
            tc.tile_pool(name="oproj", bufs=1) as oproj,
            tc.tile_pool(name="oproj2", bufs=2) as oproj2,
            tc.tile_pool(name="ps", bufs=1, space="PSUM") as ps,
            tc.tile_pool(name="dram", bufs=1, space="DRAM") as dram,
        ):
            # ---- consts ----
            ident_b = consts.tile([128, 128], BF16)
            make_identity(nc, ident_b[:])
            ident_f = consts.tile([128, 128], F32)
            make_identity(nc, ident_f[:])
            ones_b = consts.tile([1, 64], BF16)
            nc.vector.memset(ones_b[:], 1.0)
            sel_sb = consts.tile([H, 128], F32)
            nc.sync.dma_start(sel_sb[:], sel[:])

            # ---- persistent SBUF ----
            qkT = persist.tile([128, 4, NTOK], BF16)   # [2 heads x 64, cb, tok]
            vT = persist.tile([128, 2, NTOK], BF16)    # pair per 64-row half
            gate_l = persist.tile([128, NTOK], F32)  # rows 0/32/64/96 hold local heads

            # ---- DRAM internal ----
            ag_in = [dram.tile([DSL, NTOK // 2], BF16, name=f"ag_in{_i}")
                     for _i in range(2)]
            ag_out = [dram.tile([D, NTOK // 2], BF16, name=f"ag_out{_i}")
                      for _i in range(2)]
            USE_SPLIT_AG = True
            ar_in = dram.tile([NTOK, H], F32)
            ar_out = dram.tile([NTOK, H], F32)
            a2a_in = [dram.tile([1024, TB], BF16, name=f"a2a_in{_i}") for _i in range(2)]
            a2a_out = [dram.tile([1024, TB], BF16, name=f"a2a_out{_i}") for _i in range(2)]

            # ---- weights preload (converted to bf16) ----
            wqk_sb = persist.tile([128, KT * 4, 128], BF16)
            wv_sb = persist.tile([128, KT * 2, 128], BF16)
            for kt in range(KT):
                wtmp = wpool.tile([128, 512], F32, tag="wtmp")
                nc.sync.dma_start(wtmp[:], w_qk[kt * 128:(kt + 1) * 128, :])
                nc.any.tensor_copy(
                    wqk_sb[:, kt * 4:(kt + 1) * 4, :].rearrange("p a b -> p (a b)"),
                    wtmp[:])
                wtmp2 = wpool.tile([128, 256], F32, tag="wtmp2")
                nc.sync.dma_start(wtmp2[:], w_v[kt * 128:(kt + 1) * 128, :])
                nc.any.tensor_copy(
                    wv_sb[:, kt * 2:(kt + 1) * 2, :].rearrange("p a b -> p (a b)"),
                    wtmp2[:])
            wr_sb = persist.tile([128, 2, H], F32)
            nc.sync.dma_start(wr_sb[:], w_r.rearrange("(a p) h -> p a h", p=128))

            # ---- phase 1: transpose x-slice, write AG input, router partials ----
            for tokb in range(32):
                xs = xph.tile([128, DSL], F32, tag="xs")
                nc.sync.dma_start(xs[:], x_sl[tokb * 128:(tokb + 1) * 128, :])
                ps_r = ps.tile([128, H], F32, tag=f"A{2 + tokb % 2}", name="ps_r")
                for db in range(2):
                    tp = ps.tile([128, 128], F32, tag=f"A{(tokb % 2) * 4 + db}", name="tp")
                    nc.tensor.transpose(tp[:], xs[:, db * 128:(db + 1) * 128], ident_f[:])
                    hi = xph.tile([128, 128], BF16, tag="hi")
                    nc.scalar.activation(hi[:], tp[:], AF.Copy)
                    xf = xph.tile([128, 128], F32, tag="xf")
                    nc.vector.tensor_copy(xf[:], tp[:])
                    agh, agf = divmod(tokb * 128, NTOK // 2)
                    nc.sync.dma_start(
                        ag_in[agh][db * 128:(db + 1) * 128, agf:agf + 128], hi[:])
                    # router partial: [tok,32] += xf.T @ w_r  (f32, exact)
                    nc.tensor.matmul(ps_r[:], xf[:], wr_sb[:, db, :],
                                     start=(db == 0), stop=(db == 1))
                rsb = xph.tile([128, H], F32, tag="rsb")
                nc.vector.tensor_copy(rsb[:], ps_r[:])
                nc.sync.dma_start(ar_in[tokb * 128:(tokb + 1) * 128, :], rsb[:])
                if tokb == 31 or (USE_SPLIT_AG and tokb == 15):
                    for half in ([tokb // 16] if USE_SPLIT_AG else [0, 1]):
                        nc.gpsimd.collective_compute(
                            "AllGather", OP.bypass, replica_groups=[list(range(NCORES))],
                            ins=[ag_in[half].opt()], outs=[ag_out[half].opt()])

            nc.gpsimd.collective_compute(
                "AllReduce", OP.add, replica_groups=[list(range(NCORES))],
                ins=[ar_in.opt()], outs=[ar_out.opt()])

            # ---- phase 2: unified QKV^T pass ----
            for tb in range(NTB):
                tsl = slice(tb * TB, (tb + 1) * TB)
                rh = []
                for half in range(2):
                    r = stream.tile([128, 8, TB], BF16, tag="rhs", name=f"rhs{half}")
                    for kk in range(8):
                        kt = half * 8 + kk
                        aghalf, aghoff = divmod(tb * TB, NTOK // 2)
                        nc.sync.dma_start(
                            r[:, kk, :],
                            ag_out[aghalf][kt * 128:(kt + 1) * 128, aghoff:aghoff + TB])
                    rh.append(r)
                cs_c = cspool.tile([128, TB], F32, tag="cs_c")
                nc.sync.dma_start(cs_c[:], cos4[:, tsl])
                cs_s = cspool.tile([128, TB], F32, tag="cs_s")
                nc.sync.dma_start(cs_s[:], ssin4[:, tsl])
                ps_qk = [ps.tile([128, TB], F32, tag=f"A{cb}", name=f"ps_qk{cb}")
                         for cb in range(4)]
                ps_v = [ps.tile([128, TB], F32, tag=f"A{4 + vb}", name=f"ps_v{vb}")
                        for vb in range(2)]
                for kt in range(KT):
                    rhs = rh[kt // 8][:, kt % 8, :]
                    for cb in range(4):
                        nc.tensor.matmul(ps_qk[cb][:], wqk_sb[:, kt * 4 + cb, :],
                                         rhs, start=(kt == 0), stop=(kt == KT - 1))
                    for vb in range(2):
                        nc.tensor.matmul(ps_v[vb][:], wv_sb[:, kt * 2 + vb, :],
                                         rhs, start=(kt == 0), stop=(kt == KT - 1))
                # RoPE epilogue on the 4 qk blocks
                for cb in range(4):
                    csb = rope.tile([128, TB], BF16, tag="C")
                    nc.scalar.activation(csb[:], ps_qk[cb][:], AF.Copy)
                    swp = rope.tile([128, TB], BF16, tag="S")
                    for g in range(4):
                        sg = g ^ 1
                        nc.sync.dma_start(swp[g * 32:(g + 1) * 32, :],
                                          csb[sg * 32:(sg + 1) * 32, :])
                    t1 = rope.tile([128, TB], F32, tag="T1")
                    nc.vector.tensor_tensor(t1[:], ps_qk[cb][:], cs_c[:], OP.mult)
                    t2 = rope.tile([128, TB], F32, tag="T2")
                    nc.vector.tensor_tensor(t2[:], swp[:], cs_s[:], OP.mult)
                    nc.vector.tensor_tensor(qkT[:, cb, tsl], t1[:], t2[:], OP.add)
                for vb in range(2):
                    nc.scalar.activation(vT[:, vb, tsl], ps_v[vb][:], AF.Copy)

            # ---- phase 3: router softmax + top-4 + gate rows ----
            e = persist.tile([128, 32, H], F32)
            for tokb in range(32):
                nc.sync.dma_start(e[:, tokb, :], ar_out[tokb * 128:(tokb + 1) * 128, :])
            rmax = rt.tile([128, 32], F32, tag="rmax")
            nc.vector.tensor_reduce(rmax[:], e[:], axis=mybir.AxisListType.X, op=OP.max)
            nc.vector.tensor_tensor(e[:], e[:], rmax[:, :, None].to_broadcast((128, 32, H)),
                                    OP.subtract)
            nc.scalar.activation(e[:].rearrange("p a h -> p (a h)"),
                                 e[:].rearrange("p a h -> p (a h)"), AF.Exp)
            ssum = rt.tile([128, 32], F32, tag="ssum")
            nc.vector.tensor_reduce(ssum[:], e[:], axis=mybir.AxisListType.X, op=OP.add)
            rs = rt.tile([128, 32], F32, tag="rs")
            nc.vector.reciprocal(rs[:], ssum[:])
            ecur = persist.tile([128, 32, H], F32)
            nc.vector.tensor_copy(ecur[:], e[:])
            ge = rt.tile([128, 32, H], BF16, tag="geb", name="ge")
            for it in range(4):
                m = rt.tile([128, 32], F32, tag="m")
                nc.vector.tensor_reduce(m[:], ecur[:], axis=mybir.AxisListType.X, op=OP.max)
                nc.vector.tensor_tensor(ge[:], ecur[:],
                                        m[:, :, None].to_broadcast((128, 32, H)), OP.is_ge)
                nc.vector.scalar_tensor_tensor(ecur[:], ge[:], -1e9, ecur[:],
                                               OP.mult, OP.add)
            mask = rt.tile([128, 32, H], BF16, tag="geb", name="mask")
            nc.vector.tensor_scalar(mask[:], ecur[:], -1e6, None, OP.is_lt)
            gate = e
            nc.vector.tensor_tensor(gate[:], e[:], mask[:], OP.mult)
            nc.vector.tensor_tensor(gate[:], gate[:],
                                    rs[:, :, None].to_broadcast((128, 32, H)), OP.mult)
            # extract the 4 local head rows (sel is the per-core one-hot [32, 4])
            for tokb in range(32):
                gt_ps = ps.tile([H, 128], F32, tag="A6", name="gt_ps")
                nc.tensor.transpose(gt_ps[:], gate[:, tokb, :], ident_f[:])
                gt_sb = rt.tile([H, 128], F32, tag="gt_sb")
                nc.vector.tensor_copy(gt_sb[:], gt_ps[:])
                g4_ps = ps.tile([128, 128], F32, tag="A7", name="g4_ps")
                nc.tensor.matmul(g4_ps[:], sel_sb[:], gt_sb[:], start=True, stop=True)
                for l in range(HL):
                    nc.vector.tensor_copy(
                        gate_l[32 * l:32 * l + 1, tokb * 128:(tokb + 1) * 128],
                        g4_ps[32 * l:32 * l + 1, :])

            # ---- phase 4: attention per (head-pair, batch) ----
            for hp in range(2):
                for b in range(B):
                    bt = b * T
                    va = [att.tile([128, 8, 72], BF16, tag=f"va{hl}", name=f"va{hl}")
                          for hl in range(2)]
                    for hl in range(2):
                        base = hl * 64
                        idn = ident_b[base:base + 64, base:base + 64]
                        for tkb in range(8):
                            vps = ps.tile([128, 64], BF16, tag="A6", name="vps")
                            nc.tensor.transpose(
                                vps[:],
                                vT[base:base + 64, hp, bt + tkb * 128:bt + (tkb + 1) * 128],
                                idn)
                            nc.any.tensor_copy(va[hl][:, tkb, 0:64], vps[:])
                            nc.vector.memset(va[hl][:, tkb, 64:65], 1.0)
                    for tqt in range(2):
                        qsl = slice(bt + tqt * TB, bt + (tqt + 1) * TB)
                        ntk = 4 + 4 * tqt
                        o_ps = [ps.tile([65, TB], F32, tag=f"A{4 + hl}", name=f"o_ps{hl}")
                                for hl in range(2)]
                        for tkb in range(ntk):
                            ksl = slice(bt + tkb * 128, bt + (tkb + 1) * 128)
                            s_ps = [ps.tile([128, TB], F32, tag=f"A{2 * hl + tkb % 2}",
                                            name=f"s_ps{hl}") for hl in range(2)]
                            nc.tensor.matmul(s_ps[0][:], qkT[0:64, 2 + hp, ksl],
                                             qkT[0:64, hp, qsl], start=True, stop=True,
                                             tile_position=(0, 0))
                            nc.tensor.matmul(s_ps[1][:], qkT[64:128, 2 + hp, ksl],
                                             qkT[64:128, hp, qsl], start=True, stop=True,
                                             tile_position=(64, 0))
                            dd = tqt * 4 - tkb
                            for hl in range(2):
                                p_sb = att.tile([128, TB], BF16, tag=f"p{hl}", name=f"p{hl}")
                                if dd >= 1:
                                    nc.scalar.activation(p_sb[:], s_ps[hl][:], AF.Exp,
                                                         scale=0.125)
                                else:
                                    off = -dd * 128
                                    if off > 0:
                                        nc.vector.memset(p_sb[:, 0:off], 0.0)
                                    nc.scalar.activation(p_sb[:, off:TB], s_ps[hl][:, off:TB],
                                                         AF.Exp, scale=0.125)
                                    nc.gpsimd.affine_select(
                                        out=p_sb[:, off:off + 128], in_=p_sb[:, off:off + 128],
                                        compare_op=OP.is_ge, fill=0.0,
                                        base=0, pattern=[[1, 128]], channel_multiplier=-1)
                                nc.tensor.matmul(o_ps[hl][:], va[hl][:, tkb, 0:65], p_sb[:],
                                                 start=(tkb == 0), stop=(tkb == ntk - 1))
                        for hl in range(2):
                            l = 2 * hp + hl
                            grow = att.tile([1, TB], F32, tag="grow")
                            nc.vector.tensor_copy(grow[:], gate_l[32 * l:32 * l + 1, qsl])
                            dn_sb = att.tile([1, TB], F32, tag="dn_sb")
                            nc.scalar.activation(dn_sb[:], o_ps[hl][64:65, :], AF.Copy)
                            recip = att.tile([1, TB], F32, tag="recip")
                            nc.vector.reciprocal_approx_fast(recip[:], dn_sb[:])
                            scale_sb = att.tile([1, TB], BF16, tag="scale")
                            nc.vector.tensor_tensor(scale_sb[:], grow[:], recip[:], OP.mult)
                            bc_ps = ps.tile([64, TB], F32, tag="A6", name="bc_ps")
                            nc.tensor.matmul(bc_ps[:], ones_b[:], scale_sb[:],
                                             start=True, stop=True)
                            bc_sb = att.tile([64, TB], BF16, tag="bc_sb")
                            nc.scalar.activation(bc_sb[:], bc_ps[:], AF.Copy)
                            oT = att.tile([64, TB], BF16, tag="oT_sb")
                            nc.vector.tensor_tensor(oT[:], o_ps[hl][0:64, :], bc_sb[:], OP.mult)
                            j = 2 * b + tqt
                            nc.sync.dma_start(
                                a2a_in[hp][j * 128 + hl * 64:j * 128 + (hl + 1) * 64, :],
                                oT[:])
                nc.gpsimd.collective_compute(
                    "AllToAll", OP.bypass, replica_groups=[list(range(NCORES))],
                    ins=[a2a_in[hp].opt()], outs=[a2a_out[hp].opt()])

            # ---- phase 5: output projection on my token shard ----
            for hp in range(2):
                rcv = [oproj.tile([128, TB], BF16, tag=f"rcv{i}", name=f"rcv{i}")
                       for i in range(8)]
                for i in range(8):
                    nc.sync.dma_start(rcv[i][:], a2a_out[hp][i * 128:(i + 1) * 128, :])
                wo = [oproj.tile([128, 4, 512], BF16, tag=f"wo{i}", name=f"wo{i}")
                      for i in range(8)]
                for i in range(8):
                    for nw in range(8):
                        wof = oproj2.tile([128, 256], F32, tag="wof")
                        nc.sync.dma_start(
                            wof[:], w_out[i * 256 + hp * 128:i * 256 + (hp + 1) * 128,
                                          nw * 256:(nw + 1) * 256])
                        nc.any.tensor_copy(
                            wo[i][:, nw // 2, (nw % 2) * 256:(nw % 2 + 1) * 256], wof[:])
                ngrp = 4 if hp == 1 else 2
                ntags = ["A2", "A3", "A6", "A7"] if hp == 1 else ["A7", "A3"]
                for m_ in range(4):
                    for ng in range(4 // ngrp):
                        ops_t = [ps.tile([128, 512], F32, tag=ntags[nn], name=f"op{nn}")
                                 for nn in range(ngrp)]
                        for i in range(8):
                            for nn in range(ngrp):
                                n = ng * ngrp + nn
                                nc.tensor.matmul(
                                    ops_t[nn][:], rcv[i][:, m_ * 128:(m_ + 1) * 128],
                                    wo[i][:, n, :], start=(i == 0), stop=(i == 7))
                        for nn in range(ngrp):
                            n = ng * ngrp + nn
                            ostage = oproj2.tile([128, 512], F32, tag="ostage")
                            nc.vector.tensor_copy(ostage[:], ops_t[nn][:])
                            r0 = slice(m_ * 128, (m_ + 1) * 128)
                            c0 = slice(n * 512, (n + 1) * 512)
                            if hp == 0:
                                nc.sync.dma_start(out[r0, c0], ostage[:])
                            else:
                                nc.gpsimd.dma_start(out[r0, c0], ostage[:],
                                                    accum_op=OP.add)

    nc.compile()
    return nc


_CACHE = {}


def _get_nc():
    if "nc" not in _CACHE:
        _CACHE["nc"] = _build()
    return _CACHE["nc"]


def _host_inputs(x, w_router, w_qkv, w_out):
    x2 = np.ascontiguousarray(np.asarray(x, dtype=np.float32).reshape(NTOK, D))
    w_qkv = np.asarray(w_qkv, dtype=np.float32)
    w_router = np.asarray(w_router, dtype=np.float32)
    w_out = np.ascontiguousarray(np.asarray(w_out, dtype=np.float32))

    # RoPE tables, de-interleaved layout
    invf = 1.0 / (ROPE_BASE ** (np.arange(0, DH, 2, dtype=np.float32) / DH))  # [32]
    tt = np.arange(NTOK, dtype=np.float32) % T
    ang = tt[None, :] * invf[:, None]
    cos1 = np.cos(ang).astype(np.float32)
    sin1 = np.sin(ang).astype(np.float32)
    cos4 = np.ascontiguousarray(np.tile(cos1, (4, 1)).astype(np.float32))
    ssin4 = np.ascontiguousarray(
        np.concatenate([-sin1, sin1, -sin1, sin1], axis=0).astype(np.float32))

    in_maps = []
    for c in range(NCORES):
        heads = [4 * c + i for i in range(HL)]

        def deint(h, base):
            cols = np.arange(h * DH, (h + 1) * DH)
            return np.concatenate([base + cols[0::2], base + cols[1::2]])

        qk_cols = np.concatenate(
            [deint(heads[0], 0), deint(heads[1], 0),
             deint(heads[2], 0), deint(heads[3], 0),
             deint(heads[0], D), deint(heads[1], D),
             deint(heads[2], D), deint(heads[3], D)])
        v_cols = np.concatenate([2 * D + np.arange(h * DH, (h + 1) * DH) for h in heads])
        sel_np = np.zeros((H, 128), dtype=np.float32)
        for l in range(HL):
            sel_np[4 * c + l, 32 * l] = 1.0
        in_maps.append({
            "x_sl": np.ascontiguousarray(x2[:, c * DSL:(c + 1) * DSL]),
            "w_qk": np.ascontiguousarray(w_qkv[:, qk_cols]),
            "w_v": np.ascontiguousarray(w_qkv[:, v_cols]),
            "w_r": np.ascontiguousarray(w_router[c * DSL:(c + 1) * DSL, :]),
            "w_out": w_out,
            "cos4": cos4,
            "ssin4": ssin4,
            "sel": sel_np,
        })
    return in_maps


def run(x, w_router, w_qkv, w_out, trace=False):
    nc = _get_nc()
    in_maps = _host_inputs(x, w_router, w_qkv, w_out)
    res = run_bass_kernel_spmd(nc, in_maps, core_ids=list(range(NCORES)), trace=trace)
    shards = [res.results[c]["out"] for c in range(NCORES)]
    full = np.concatenate(shards, axis=0).reshape(B, T, D).astype(np.float32)
    return full, res


def kernel(x, w_router, w_qkv, w_out):
    full, _ = run(x, w_router, w_qkv, w_out, trace=False)
    return full
